# revision 3
# baseline (speedup 1.0000x reference)
"""FFM layer (linear + field-aware FM interaction) on 8 Trainium2 cores.

Sharding: row-parallel GEMM over the feature axis. Core c holds a
13056-feature stripe of inputs^T ([13056, 1024]) and of the combined
weight matrix G = [v.reshape(F, 312) | w] ([13056, 313]). Each core
computes its partial inputs_c^T.T @ G_c -> [1024, 313] with fp32
(float32r PE mode) matmuls accumulated in PSUM over 102 k-tiles.
The host sums the 8 partials and applies the cheap FM epilogue
(sum-square identity) in fp64, returning [1024, 1] fp32.
"""

import numpy as np

B = 1024
F = 104013
FIELD = 39
K = 8
NV = FIELD * K          # 312 interaction columns
NL = NV                 # linear column index
NK = NV + 2             # + linear column + 1 zero pad col (fp32r needs even N)
N_CORES = 8
KT = 102                # 128-row k-tiles per core
FPC = KT * 128          # 13056 padded features per core
CH = 6                  # k-tiles per DMA chunk (102 = 17 * 6)
DMA_ENGINE = "sync"     # "sync" (HWDGE) or "gpsimd" (SWDGE)

_nc = None
last_exec_time_ns = None


def _build():
    from concourse import bass, mybir, tile, bacc

    nc = bacc.Bacc("TRN2", num_devices=N_CORES)
    f32 = mybir.dt.float32
    f32r = mybir.dt.float32r

    xt = nc.dram_tensor("xt", [FPC, B], f32r, kind="ExternalInput")
    g = nc.dram_tensor("g", [FPC, NK], f32r, kind="ExternalInput")
    out = nc.dram_tensor("out", [B, NK], f32, kind="ExternalOutput")

    xt_r = xt.rearrange("(t p) m -> p t m", p=128)  # [128, KT, B]
    g_r = g.rearrange("(t p) n -> p t n", p=128)    # [128, KT, NK]

    with tile.TileContext(nc) as tc:
        with (
            tc.tile_pool(name="xt", bufs=3) as xt_pool,
            tc.tile_pool(name="g", bufs=3) as g_pool,
            tc.tile_pool(name="acc", bufs=1, space=bass.MemorySpace.PSUM) as psum_pool,
            tc.tile_pool(name="o", bufs=2) as out_pool,
        ):
            n_b = B // 128
            accs = [
                psum_pool.tile([128, NK], f32, tag=f"acc{b}", name=f"acc{b}")
                for b in range(n_b)
            ]
            dma = nc.sync if DMA_ENGINE == "sync" else nc.gpsimd
            for kc in range(0, KT, CH):
                n = min(CH, KT - kc)
                xt_t = xt_pool.tile([128, n, B], f32r, tag="xt", name=f"xt{kc}")
                dma.dma_start(xt_t[:], xt_r[:, kc : kc + n, :])
                g_t = g_pool.tile([128, n, NK], f32r, tag="g", name=f"gt{kc}")
                dma.dma_start(g_t[:], g_r[:, kc : kc + n, :])
                for i in range(n):
                    k = kc + i
                    for b in range(n_b):
                        nc.tensor.matmul(
                            accs[b][:],
                            xt_t[:, i, b * 128 : (b + 1) * 128],
                            g_t[:, i, :],
                            start=(k == 0),
                            stop=(k == KT - 1),
                        )
            for b in range(n_b):
                o = out_pool.tile([128, NK], f32, tag="o", name=f"ot{b}")
                nc.vector.tensor_copy(o[:], accs[b][:])
                dma.dma_start(out[b * 128 : (b + 1) * 128, :], o[:])
    nc.compile()
    return nc


def _get_nc():
    global _nc
    if _nc is None:
        _nc = _build()
    return _nc


def kernel(inputs, w0, w, v, _trace=False):
    global last_exec_time_ns
    from concourse.bass_utils import run_bass_kernel_spmd

    inputs = np.asarray(inputs, dtype=np.float32)
    w0 = np.asarray(w0, dtype=np.float32)
    w = np.asarray(w, dtype=np.float32)
    v = np.asarray(v, dtype=np.float32)

    # G = [v | w] : [F, 313], zero-padded to 8 * 13056 rows
    G = np.zeros((N_CORES * FPC, NK), dtype=np.float32)
    G[:F, :NV] = v.reshape(F, NV)
    G[:F, NL] = w[:, 0]
    # inputs^T, zero-padded the same way
    XT = np.zeros((N_CORES * FPC, B), dtype=np.float32)
    XT[:F] = inputs.T

    in_maps = [
        {"xt": XT[c * FPC : (c + 1) * FPC], "g": G[c * FPC : (c + 1) * FPC]}
        for c in range(N_CORES)
    ]
    nc = _get_nc()
    res = run_bass_kernel_spmd(nc, in_maps, list(range(N_CORES)), trace=_trace)
    last_exec_time_ns = res.exec_time_ns

    total = np.zeros((B, NK), dtype=np.float64)
    for c in range(N_CORES):
        total += res.results[c]["out"]

    field_f = total[:, :NV].reshape(B, FIELD, K)
    linear = total[:, NL] + np.float64(w0[0])
    s = field_f.sum(axis=1)                                     # [B, K]
    inter = 0.5 * ((s * s).sum(axis=-1) - (field_f * field_f).sum(axis=(1, 2)))
    return (linear + inter)[:, None].astype(np.float32)


# revision 5
# speedup vs baseline: 1.0308x; 1.0308x over previous
"""FFM layer (linear + field-aware FM interaction) on 8 Trainium2 cores.

Sharding: row-parallel GEMM over the feature axis. Core c holds a
13056-feature stripe of inputs^T ([13056, 1024]) and of the combined
weight matrix G = [v.reshape(F, 312) | w] ([13056, 313]). Each core
computes its partial inputs_c^T.T @ G_c -> [1024, 313] with fp32
(float32r PE mode) matmuls accumulated in PSUM over 102 k-tiles.
The host sums the 8 partials and applies the cheap FM epilogue
(sum-square identity) in fp64, returning [1024, 1] fp32.
"""

import numpy as np

B = 1024
F = 104013
FIELD = 39
K = 8
NV = FIELD * K          # 312 interaction columns
NL = NV                 # linear column index
NK = NV + 2             # + linear column + 1 zero pad col (fp32r needs even N)
N_CORES = 8
KT = 102                # 128-row k-tiles per core
FPC = KT * 128          # 13056 padded features per core
CH = 3                  # k-tiles per DMA chunk (102 = 34 * 3)
DMA_ENGINE = "sync"     # "sync" (HWDGE) or "gpsimd" (SWDGE)

_nc = None
last_exec_time_ns = None


def _build():
    from concourse import bass, mybir, tile, bacc

    nc = bacc.Bacc("TRN2", num_devices=N_CORES)
    f32 = mybir.dt.float32
    f32r = mybir.dt.float32r

    xt = nc.dram_tensor("xt", [FPC, B], f32r, kind="ExternalInput")
    g = nc.dram_tensor("g", [FPC, NK], f32r, kind="ExternalInput")
    out = nc.dram_tensor("out", [B, NK], f32, kind="ExternalOutput")

    xt_r = xt.rearrange("(t p) m -> p t m", p=128)  # [128, KT, B]
    g_r = g.rearrange("(t p) n -> p t n", p=128)    # [128, KT, NK]

    with tile.TileContext(nc) as tc:
        with (
            tc.tile_pool(name="xt", bufs=6) as xt_pool,
            tc.tile_pool(name="g", bufs=6) as g_pool,
            tc.tile_pool(name="acc", bufs=1, space=bass.MemorySpace.PSUM) as psum_pool,
            tc.tile_pool(name="o", bufs=2) as out_pool,
        ):
            n_b = B // 128
            accs = [
                psum_pool.tile([128, NK], f32, tag=f"acc{b}", name=f"acc{b}")
                for b in range(n_b)
            ]
            dma = nc.sync if DMA_ENGINE == "sync" else nc.gpsimd
            for kc in range(0, KT, CH):
                n = min(CH, KT - kc)
                xt_t = xt_pool.tile([128, n, B], f32r, tag="xt", name=f"xt{kc}")
                dma.dma_start(xt_t[:], xt_r[:, kc : kc + n, :])
                g_t = g_pool.tile([128, n, NK], f32r, tag="g", name=f"gt{kc}")
                dma.dma_start(g_t[:], g_r[:, kc : kc + n, :])
                for i in range(n):
                    k = kc + i
                    for b in range(n_b):
                        nc.tensor.matmul(
                            accs[b][:],
                            xt_t[:, i, b * 128 : (b + 1) * 128],
                            g_t[:, i, :],
                            start=(k == 0),
                            stop=(k == KT - 1),
                        )
            for b in range(n_b):
                o = out_pool.tile([128, NK], f32, tag="o", name=f"ot{b}")
                nc.vector.tensor_copy(o[:], accs[b][:])
                dma.dma_start(out[b * 128 : (b + 1) * 128, :], o[:])
    nc.compile()
    return nc


def _get_nc():
    global _nc
    if _nc is None:
        _nc = _build()
    return _nc


def kernel(inputs, w0, w, v, _trace=False):
    global last_exec_time_ns
    from concourse.bass_utils import run_bass_kernel_spmd

    inputs = np.asarray(inputs, dtype=np.float32)
    w0 = np.asarray(w0, dtype=np.float32)
    w = np.asarray(w, dtype=np.float32)
    v = np.asarray(v, dtype=np.float32)

    # G = [v | w] : [F, 313], zero-padded to 8 * 13056 rows
    G = np.zeros((N_CORES * FPC, NK), dtype=np.float32)
    G[:F, :NV] = v.reshape(F, NV)
    G[:F, NL] = w[:, 0]
    # inputs^T, zero-padded the same way
    XT = np.zeros((N_CORES * FPC, B), dtype=np.float32)
    XT[:F] = inputs.T

    in_maps = [
        {"xt": XT[c * FPC : (c + 1) * FPC], "g": G[c * FPC : (c + 1) * FPC]}
        for c in range(N_CORES)
    ]
    nc = _get_nc()
    res = run_bass_kernel_spmd(nc, in_maps, list(range(N_CORES)), trace=_trace)
    last_exec_time_ns = res.exec_time_ns

    total = np.zeros((B, NK), dtype=np.float64)
    for c in range(N_CORES):
        total += res.results[c]["out"]

    field_f = total[:, :NV].reshape(B, FIELD, K)
    linear = total[:, NL] + np.float64(w0[0])
    s = field_f.sum(axis=1)                                     # [B, K]
    inter = 0.5 * ((s * s).sum(axis=-1) - (field_f * field_f).sum(axis=(1, 2)))
    return (linear + inter)[:, None].astype(np.float32)


# revision 6
# speedup vs baseline: 1.0322x; 1.0013x over previous
"""FFM layer (linear + field-aware FM interaction) on 8 Trainium2 cores.

Sharding: row-parallel GEMM over the feature axis. Core c holds a
13056-feature stripe of inputs^T ([13056, 1024]) and of the combined
weight matrix G = [v.reshape(F, 312) | w] ([13056, 313]). Each core
computes its partial inputs_c^T.T @ G_c -> [1024, 313] with fp32
(float32r PE mode) matmuls accumulated in PSUM over 102 k-tiles.
The host sums the 8 partials and applies the cheap FM epilogue
(sum-square identity) in fp64, returning [1024, 1] fp32.
"""

import numpy as np

B = 1024
F = 104013
FIELD = 39
K = 8
NV = FIELD * K          # 312 interaction columns
NL = NV                 # linear column index
NK = NV + 2             # + linear column + 1 zero pad col (fp32r needs even N)
N_CORES = 8
KT = 102                # 128-row k-tiles per core
FPC = KT * 128          # 13056 padded features per core
CH = 3                  # k-tiles per DMA chunk
BUFS = 6                # SBUF double-buffer depth for streamed chunks
DMA_ENGINE = "sync"     # "sync" (HWDGE) or "gpsimd" (SWDGE)

_nc = None
last_exec_time_ns = None


def _build():
    from concourse import bass, mybir, tile, bacc

    nc = bacc.Bacc("TRN2", num_devices=N_CORES)
    f32 = mybir.dt.float32
    f32r = mybir.dt.float32r

    xt = nc.dram_tensor("xt", [FPC, B], f32r, kind="ExternalInput")
    g = nc.dram_tensor("g", [FPC, NK], f32r, kind="ExternalInput")
    out = nc.dram_tensor("out", [B, NK], f32, kind="ExternalOutput")

    xt_r = xt.rearrange("(t p) m -> p t m", p=128)  # [128, KT, B]
    g_r = g.rearrange("(t p) n -> p t n", p=128)    # [128, KT, NK]

    with tile.TileContext(nc) as tc:
        with (
            tc.tile_pool(name="xt", bufs=BUFS) as xt_pool,
            tc.tile_pool(name="g", bufs=BUFS) as g_pool,
            tc.tile_pool(name="acc", bufs=1, space=bass.MemorySpace.PSUM) as psum_pool,
            tc.tile_pool(name="o", bufs=2) as out_pool,
        ):
            n_b = B // 128
            accs = [
                psum_pool.tile([128, NK], f32, tag=f"acc{b}", name=f"acc{b}")
                for b in range(n_b)
            ]
            dma = nc.sync if DMA_ENGINE == "sync" else nc.gpsimd
            for kc in range(0, KT, CH):
                n = min(CH, KT - kc)
                xt_t = xt_pool.tile([128, n, B], f32r, tag="xt", name=f"xt{kc}")
                dma.dma_start(xt_t[:], xt_r[:, kc : kc + n, :])
                g_t = g_pool.tile([128, n, NK], f32r, tag="g", name=f"gt{kc}")
                dma.dma_start(g_t[:], g_r[:, kc : kc + n, :])
                for i in range(n):
                    k = kc + i
                    for b in range(n_b):
                        nc.tensor.matmul(
                            accs[b][:],
                            xt_t[:, i, b * 128 : (b + 1) * 128],
                            g_t[:, i, :],
                            start=(k == 0),
                            stop=(k == KT - 1),
                        )
            for b in range(n_b):
                o = out_pool.tile([128, NK], f32, tag="o", name=f"ot{b}")
                nc.vector.tensor_copy(o[:], accs[b][:])
                dma.dma_start(out[b * 128 : (b + 1) * 128, :], o[:])
    nc.compile()
    return nc


def _get_nc():
    global _nc
    if _nc is None:
        _nc = _build()
    return _nc


def kernel(inputs, w0, w, v, _trace=False):
    global last_exec_time_ns
    from concourse.bass_utils import run_bass_kernel_spmd

    inputs = np.asarray(inputs, dtype=np.float32)
    w0 = np.asarray(w0, dtype=np.float32)
    w = np.asarray(w, dtype=np.float32)
    v = np.asarray(v, dtype=np.float32)

    # G = [v | w] : [F, 313], zero-padded to 8 * 13056 rows
    G = np.zeros((N_CORES * FPC, NK), dtype=np.float32)
    G[:F, :NV] = v.reshape(F, NV)
    G[:F, NL] = w[:, 0]
    # inputs^T, zero-padded the same way
    XT = np.zeros((N_CORES * FPC, B), dtype=np.float32)
    XT[:F] = inputs.T

    in_maps = [
        {"xt": XT[c * FPC : (c + 1) * FPC], "g": G[c * FPC : (c + 1) * FPC]}
        for c in range(N_CORES)
    ]
    nc = _get_nc()
    res = run_bass_kernel_spmd(nc, in_maps, list(range(N_CORES)), trace=_trace)
    last_exec_time_ns = res.exec_time_ns

    total = np.zeros((B, NK), dtype=np.float64)
    for c in range(N_CORES):
        total += res.results[c]["out"]

    field_f = total[:, :NV].reshape(B, FIELD, K)
    linear = total[:, NL] + np.float64(w0[0])
    s = field_f.sum(axis=1)                                     # [B, K]
    inter = 0.5 * ((s * s).sum(axis=-1) - (field_f * field_f).sum(axis=(1, 2)))
    return (linear + inter)[:, None].astype(np.float32)
